# revision 21
# baseline (speedup 1.0000x reference)
"""Trainium2 Bass kernel for nn_MHA_2688649527670.

Reference computes, per batch b and head h:
    Q = x Wq_h^T, K = x Wk_h^T, V = x Wv_h^T          ([S, D] each)
    Z = softmax_over_d( (Q K^T / sqrt(D)) V )

There is NO softmax between Q K^T and V, so the chain is associative:
    (Q K^T) V = x * (Wq_h^T Wk_h G Wv_h^T) / sqrt(D),   G = x^T x   ([D, D])

This collapses the O(S^2 D) attention into a [D,D] weight-chain plus one
[S,D]x[D,D] matmul per head, followed by softmax over the model dim.

Sharding: data parallel over batch (4) x tensor parallel over head-groups
(2 groups of 4 heads) = 8 cores, fully independent (no collectives).

Measured hardware facts driving the structure (this part is power-capped:
PE ~1.1GHz max; N=512 matmul ~460ns fp32r or bf16; LDWEIGHTS 395ns fp32r
vs 116ns bf16; DVE reduce ~680ns per op nearly independent of size up to
1024 elems/partition; Pool tensor_tensor ~1ns/elem; fp32r ~ 13-bit mantissa):
  * finals fp32r (bf16 finals measured 3.1e-2 error: too lossy).
  * G = x^T x fully bf16 (total error ~8e-3 vs 2e-2 budget).
  * per-(row,head) max subtract runs ON the PE as a rank-1 accumulating
    matmul: identity stationary (bf16, exact), -max broadcast over d moving.
    Softmax is shift-invariant so the bf16-rounded shift stays exact.
  * chunks processed in PAIRS sharing one 2-bank PSUM tile: one DVE max,
    one [128,1024] exp, one DVE sum, one reciprocal, one Pool multiply and
    one contiguous 256KB store per pair (fixed per-op costs dominate).
  * output stored as bf16 (0.2% quantization, halves store traffic);
    host upcasts to f32.
  * x^T / per-head Wv^T staged pre-transposed on the HOST (pure layout).
"""

import numpy as np
import ml_dtypes

import concourse.bass as bass
import concourse.bacc as bacc
import concourse.mybir as mybir
import concourse.tile as tile
from concourse.bass_utils import run_bass_kernel_spmd

B, S, D, H = 4, 2048, 128, 8
P = 128
HPC = H // 2          # heads per core (tensor parallel over 2 head groups)
NCH = S // P          # 16 s-chunks of 128 rows
NW = HPC * D          # 512 output columns per core
N_CORES = 8
SCALE = 1.0 / float(np.sqrt(D))
F32 = mybir.dt.float32
F32R = mybir.dt.float32r
BF16 = mybir.dt.bfloat16

_PROG = None  # cached compiled Bass program (same SPMD program for all cores)


def _build_program():
    nc = bacc.Bacc("TRN2", target_bir_lowering=False, debug=False,
                   num_devices=N_CORES)

    xb_d = nc.dram_tensor("xb", [P, NCH * D], BF16, kind="ExternalInput")
    xt_d = nc.dram_tensor("xt", [D, S], F32R, kind="ExternalInput")
    wq_d = nc.dram_tensor("wq", [P, HPC * D], F32R, kind="ExternalInput")
    wk_d = nc.dram_tensor("wk", [P, HPC * D], F32R, kind="ExternalInput")
    wvt_d = nc.dram_tensor("wvt", [P, HPC * D], F32R, kind="ExternalInput")
    id_d = nc.dram_tensor("ident", [P, P], BF16, kind="ExternalInput")
    out_d = nc.dram_tensor("out", [S, NW], BF16, kind="ExternalOutput")

    with tile.TileContext(nc) as tc:
        with (
            tc.tile_pool(name="ps", bufs=4, space="PSUM") as ps,
            tc.tile_pool(name="const", bufs=1) as const,
            tc.tile_pool(name="chain", bufs=2) as chain,
            tc.tile_pool(name="work", bufs=3) as work,
        ):
            # all PSUM flows through one ring of 4 two-bank pair tiles; the
            # prologue carves its small accumulators out of the first two
            # ring slots (dead before the pair loop wraps around to them)
            pro_a = ps.tile([P, 2, NW], F32, tag="y2")
            pro_b = ps.tile([P, 2, NW], F32, tag="y2")

            # PE p-state warm-up: the clock ramps only under continuous load,
            # and G lands on a cold PE otherwise. Values are never read.
            warm = const.tile([P, NW], BF16, tag="warm")
            nc.gpsimd.memset(warm, 1.0)
            for _ in range(12):
                nc.tensor.matmul(pro_a[:, 1, :], lhsT=warm[:, 0:P], rhs=warm)
            # weights are host-staged to the SBUF layout [p, h, c], so each
            # DMA is a plain [128, 512] contiguous 2D copy (2KB/partition)
            w_sb = {}
            for eng, nm, wd in ((nc.sync, "wq", wq_d), (nc.sync, "wk", wk_d),
                                (nc.scalar, "wvt", wvt_d)):
                t = const.tile([P, HPC, D], F32R, tag=f"{nm}_sb", name=f"{nm}_sb")
                eng.dma_start(t[:].rearrange("p h c -> p (h c)"), wd.ap())
                w_sb[nm] = t
            ident = const.tile([P, P], BF16, tag="ident")
            nc.scalar.dma_start(ident, id_d.ap())
            # x (bf16, for G) rides ALONE on the sync queue: DMAs sharing a
            # queue complete together, and G gates the whole prologue
            x_bf = const.tile([P, NCH, D], BF16, tag="x_bf")
            nc.sync.dma_start(x_bf[:].rearrange("p n c -> p (n c)"), xb_d.ap())
            # x^T for the finals, last on the scalar queue (needed latest)
            xt_sb = const.tile([P, S], F32R, tag="xt_sb")
            nc.scalar.dma_start(xt_sb, xt_d.ap())

            # preload the Exp activation table while DMAs stream
            pre_e = const.tile([P, 1], BF16, tag="pre_e")
            nc.scalar.activation(pre_e, ident[:, 0:1],
                                 mybir.ActivationFunctionType.Exp)

            # ---- chain part 1: P0T_h = (Wq_h^T Wk_h)^T (independent of G) ---
            p0t_all = const.tile([P, HPC, P], F32R, tag="p0t_all")
            for h in range(HPC):
                nc.tensor.matmul(pro_b[:, 0, h * P:(h + 1) * P],
                                 lhsT=w_sb["wk"][:, h, :],
                                 rhs=w_sb["wq"][:, h, :])
            nc.vector.tensor_copy(
                p0t_all[:].rearrange("p h c -> p (h c)"), pro_b[:, 0, :])

            # ---- G = x^T x in bf16 (error budget is ample; 3.4x faster) ----
            g_ps = pro_a[:, 0, 0:P]
            for i in range(NCH):
                nc.tensor.matmul(g_ps, lhsT=x_bf[:, i, :], rhs=x_bf[:, i, :],
                                 start=(i == 0), stop=(i == NCH - 1))
            g_sb = const.tile([P, P], F32R, tag="g_sb")
            nc.vector.tensor_copy(g_sb, g_ps)

            # ---- chain part 2: UT for all heads in ONE N=512 matmul,
            #      then M_h = UT_h^T WvT_h / sqrt(D) ----
            ut_ps = pro_a[:, 1, :]
            nc.tensor.matmul(ut_ps, lhsT=g_sb,
                             rhs=p0t_all[:].rearrange("p h c -> p (h c)"))
            ut_sb = chain.tile([P, HPC, P], F32R, tag="ut_sb")
            nc.vector.tensor_copy(ut_sb, ut_ps)

            m_all = const.tile([P, HPC, D], F32R, tag="m_all")
            for h in range(HPC):
                nc.tensor.matmul(pro_b[:, 1, h * P:(h + 1) * P],
                                 lhsT=ut_sb[:, h, :],
                                 rhs=w_sb["wvt"][:, h, :])
            nc.scalar.mul(m_all[:].rearrange("p h d -> p (h d)"),
                          pro_b[:, 1, :], SCALE)

            # ---- main loop over chunk PAIRS (one 2-bank PSUM tile each) ----
            m_flat = m_all[:].rearrange("p h d -> p (h d)")
            for j in range(NCH // 2):
                y2 = ps.tile([P, 2, NW], F32, tag="y2")
                for k in range(2):
                    i = 2 * j + k
                    nc.tensor.matmul(y2[:, k, :],
                                     lhsT=xt_sb[:, i * P:(i + 1) * P],
                                     rhs=m_flat, start=True, stop=False,
                                     skip_group_check=True)

                # one -max reduce for both chunks (all 8 (chunk,head) groups)
                negmax2 = work.tile([P, 2, HPC], BF16, tag="negmax2")
                nc.vector.reduce_max(
                    out=negmax2,
                    in_=y2[:].rearrange("p c (h d) -> p c h d", h=HPC),
                    axis=mybir.AxisListType.X, negate=True)

                # rank-1 subtract per chunk: y[s,(h,d)] += I * (-max bcast)
                for k in range(2):
                    nc.tensor.matmul(
                        y2[:, k, :].rearrange("p (h d) -> p h d", h=HPC),
                        lhsT=ident,
                        rhs=negmax2[:, k, :, None].to_broadcast((P, HPC, D)),
                        start=False, stop=True, skip_group_check=True)

                # one exp over both chunks (bias-free), bf16 out
                e2 = work.tile([P, 2, HPC, D], BF16, tag="e2")
                nc.scalar.activation(
                    e2, y2[:].rearrange("p c (h d) -> p c h d", h=HPC),
                    mybir.ActivationFunctionType.Exp)

                sums2 = work.tile([P, 2, HPC], F32, tag="sums2")
                nc.vector.reduce_sum(out=sums2, in_=e2,
                                     axis=mybir.AxisListType.X)
                rsum2 = work.tile([P, 2, HPC], F32, tag="rsum2")
                nc.vector.reciprocal(rsum2, sums2)

                o2 = work.tile([P, 2, HPC, D], BF16, tag="o2")
                # scalar engine takes two (chunk,head) slices of the multiply
                # (per-partition scale = rsum2[:,c,h]); Pool does the rest
                for c, h in ((0, 0), (1, 0)):
                    nc.scalar.mul(o2[:, c, h, :], e2[:, c, h, :],
                                  rsum2[:, c, h:h + 1])
                nc.gpsimd.tensor_tensor(
                    o2[:, :, 1:, :], e2[:, :, 1:, :],
                    rsum2[:, :, 1:, None].to_broadcast((P, 2, HPC - 1, D)),
                    mybir.AluOpType.mult)
                # one contiguous 256KB store per pair
                nc.sync.dma_start(
                    out_d.ap()[2 * j * P:(2 * j + 2) * P, :]
                        .rearrange("(c p) w -> p c w", p=P),
                    o2[:].rearrange("p c h d -> p c (h d)"))

    nc.compile()
    return nc


def _get_program():
    global _PROG
    if _PROG is None:
        _PROG = _build_program()
    return _PROG


def _make_in_maps(x, W_q, W_k, W_v):
    # pure host-side LAYOUT staging (transpose/slice only, no arithmetic)
    wvt_full = np.ascontiguousarray(
        W_v.reshape(H, D, D).transpose(0, 2, 1)).reshape(H * D, D)
    ident = np.eye(P, dtype=np.float32).astype(ml_dtypes.bfloat16)
    in_maps = []
    for core in range(N_CORES):
        b, hg = core // 2, core % 2
        sl = slice(hg * HPC * D, (hg + 1) * HPC * D)
        def stage(w):  # [(h p), c] -> [p, (h c)] to match the SBUF tile
            return np.ascontiguousarray(
                w.reshape(HPC, P, D).transpose(1, 0, 2).reshape(P, HPC * D))
        in_maps.append({
            "xb": np.ascontiguousarray(
                x[b].reshape(P, NCH * D).astype(ml_dtypes.bfloat16)),
            "xt": np.ascontiguousarray(x[b].T),
            "wq": stage(W_q[sl]),
            "wk": stage(W_k[sl]),
            "wvt": stage(wvt_full[sl]),
            "ident": ident,
        })
    return in_maps


def run(x, W_q, W_k, W_v, trace=False, **spmd_kwargs):
    """Run on 8 NeuronCores; returns (Z, BassKernelResults)."""
    nc = _get_program()
    in_maps = _make_in_maps(np.asarray(x, np.float32), np.asarray(W_q, np.float32),
                            np.asarray(W_k, np.float32), np.asarray(W_v, np.float32))
    res = run_bass_kernel_spmd(nc, in_maps, core_ids=list(range(N_CORES)),
                               trace=trace, **spmd_kwargs)
    Z = np.empty((B, H, S, D), np.float32)
    for core in range(N_CORES):
        b, hg = core // 2, core % 2
        o = np.asarray(res.results[core]["out"]).astype(np.float32)
        Z[b, hg * HPC:(hg + 1) * HPC] = o.reshape(S, HPC, D).transpose(1, 0, 2)
    return Z, res


def kernel(x, W_q, W_k, W_v):
    Z, _ = run(x, W_q, W_k, W_v, trace=False)
    return Z


# revision 22
# speedup vs baseline: 1.0678x; 1.0678x over previous
"""Trainium2 Bass kernel for nn_MHA_2688649527670.

Reference computes, per batch b and head h:
    Q = x Wq_h^T, K = x Wk_h^T, V = x Wv_h^T          ([S, D] each)
    Z = softmax_over_d( (Q K^T / sqrt(D)) V )

There is NO softmax between Q K^T and V, so the chain is associative:
    (Q K^T) V = x * (Wq_h^T Wk_h G Wv_h^T) / sqrt(D),   G = x^T x   ([D, D])

This collapses the O(S^2 D) attention into a [D,D] weight-chain plus one
[S,D]x[D,D] matmul per head, followed by softmax over the model dim.

Sharding: data parallel over batch (4) x tensor parallel over head-groups
(2 groups of 4 heads) = 8 cores, fully independent (no collectives).

Measured hardware facts driving the structure (this part is power-capped:
PE ~1.1GHz max; N=512 matmul ~460ns fp32r or bf16; LDWEIGHTS 395ns fp32r
vs 116ns bf16; DVE reduce ~680ns per op nearly independent of size up to
1024 elems/partition; Pool tensor_tensor ~1ns/elem; fp32r ~ 13-bit mantissa):
  * finals fp32r (bf16 finals measured 3.1e-2 error: too lossy).
  * G = x^T x fully bf16 (total error ~8e-3 vs 2e-2 budget).
  * per-(row,head) max subtract runs ON the PE as a rank-1 accumulating
    matmul: identity stationary (bf16, exact), -max broadcast over d moving.
    Softmax is shift-invariant so the bf16-rounded shift stays exact.
  * chunks processed in PAIRS sharing one 2-bank PSUM tile: one DVE max,
    one [128,1024] exp, one DVE sum, one reciprocal, one Pool multiply and
    one contiguous 256KB store per pair (fixed per-op costs dominate).
  * output stored as bf16 (0.2% quantization, halves store traffic);
    host upcasts to f32.
  * x^T / per-head Wv^T staged pre-transposed on the HOST (pure layout).
"""

import numpy as np
import ml_dtypes

import concourse.bass as bass
import concourse.bacc as bacc
import concourse.mybir as mybir
import concourse.tile as tile
from concourse.bass_utils import run_bass_kernel_spmd

B, S, D, H = 4, 2048, 128, 8
P = 128
HPC = H // 2          # heads per core (tensor parallel over 2 head groups)
NCH = S // P          # 16 s-chunks of 128 rows
NW = HPC * D          # 512 output columns per core
N_CORES = 8
SCALE = 1.0 / float(np.sqrt(D))
F32 = mybir.dt.float32
F32R = mybir.dt.float32r
BF16 = mybir.dt.bfloat16

_PROG = None  # cached compiled Bass program (same SPMD program for all cores)


def _build_program():
    nc = bacc.Bacc("TRN2", target_bir_lowering=False, debug=False,
                   num_devices=N_CORES)

    xb_d = nc.dram_tensor("xb", [P, NCH * D], BF16, kind="ExternalInput")
    xt_d = nc.dram_tensor("xt", [D, S], F32R, kind="ExternalInput")
    wq_d = nc.dram_tensor("wq", [P, HPC * D], F32R, kind="ExternalInput")
    wk_d = nc.dram_tensor("wk", [P, HPC * D], F32R, kind="ExternalInput")
    wvt_d = nc.dram_tensor("wvt", [P, HPC * D], F32R, kind="ExternalInput")
    id_d = nc.dram_tensor("ident", [P, P], BF16, kind="ExternalInput")
    out_d = nc.dram_tensor("out", [S, NW], BF16, kind="ExternalOutput")

    with tile.TileContext(nc) as tc:
        with (
            tc.tile_pool(name="ps", bufs=4, space="PSUM") as ps,
            tc.tile_pool(name="const", bufs=1) as const,
            tc.tile_pool(name="chain", bufs=2) as chain,
            tc.tile_pool(name="work", bufs=3) as work,
        ):
            # all PSUM flows through one ring of 4 two-bank pair tiles; the
            # prologue carves its small accumulators out of the first two
            # ring slots (dead before the pair loop wraps around to them)
            pro_a = ps.tile([P, 2, NW], F32, tag="y2")
            pro_b = ps.tile([P, 2, NW], F32, tag="y2")

            # PE p-state warm-up: the clock ramps only under continuous load,
            # and G lands on a cold PE otherwise. Values are never read.
            warm = const.tile([P, NW], BF16, tag="warm")
            nc.gpsimd.memset(warm, 1.0)
            for _ in range(12):
                nc.tensor.matmul(pro_a[:, 1, :], lhsT=warm[:, 0:P], rhs=warm)
            # weights are host-staged to the SBUF layout [p, h, c], so each
            # DMA is a plain [128, 512] contiguous 2D copy (2KB/partition)
            w_sb = {}
            for eng, nm, wd in ((nc.scalar, "wq", wq_d), (nc.scalar, "wk", wk_d),
                                (nc.scalar, "wvt", wvt_d)):
                t = const.tile([P, HPC, D], F32R, tag=f"{nm}_sb", name=f"{nm}_sb")
                eng.dma_start(t[:].rearrange("p h c -> p (h c)"), wd.ap())
                w_sb[nm] = t
            ident = const.tile([P, P], BF16, tag="ident")
            nc.scalar.dma_start(ident, id_d.ap())
            # x (bf16, for G) rides ALONE on the sync queue: DMAs sharing a
            # queue complete together, and G gates the whole prologue
            x_bf = const.tile([P, NCH, D], BF16, tag="x_bf")
            nc.sync.dma_start(x_bf[:].rearrange("p n c -> p (n c)"), xb_d.ap())
            # x^T for the finals, last on the scalar queue (needed latest)
            xt_sb = const.tile([P, S], F32R, tag="xt_sb")
            nc.scalar.dma_start(xt_sb, xt_d.ap())

            # preload the Exp activation table while DMAs stream
            pre_e = const.tile([P, 1], BF16, tag="pre_e")
            nc.scalar.activation(pre_e, ident[:, 0:1],
                                 mybir.ActivationFunctionType.Exp)

            # ---- chain part 1: P0T_h = (Wq_h^T Wk_h)^T (independent of G) ---
            p0t_all = const.tile([P, HPC, P], F32R, tag="p0t_all")
            for h in range(HPC):
                nc.tensor.matmul(pro_b[:, 0, h * P:(h + 1) * P],
                                 lhsT=w_sb["wk"][:, h, :],
                                 rhs=w_sb["wq"][:, h, :])
            nc.vector.tensor_copy(
                p0t_all[:].rearrange("p h c -> p (h c)"), pro_b[:, 0, :])

            # ---- G = x^T x in bf16 (error budget is ample; 3.4x faster) ----
            g_ps = pro_a[:, 0, 0:P]
            for i in range(NCH):
                nc.tensor.matmul(g_ps, lhsT=x_bf[:, i, :], rhs=x_bf[:, i, :],
                                 start=(i == 0), stop=(i == NCH - 1))
            g_sb = const.tile([P, P], F32R, tag="g_sb")
            nc.vector.tensor_copy(g_sb, g_ps)

            # ---- chain part 2: UT for all heads in ONE N=512 matmul,
            #      then M_h = UT_h^T WvT_h / sqrt(D) ----
            ut_ps = pro_a[:, 1, :]
            nc.tensor.matmul(ut_ps, lhsT=g_sb,
                             rhs=p0t_all[:].rearrange("p h c -> p (h c)"))
            ut_sb = chain.tile([P, HPC, P], F32R, tag="ut_sb")
            nc.vector.tensor_copy(ut_sb, ut_ps)

            m_all = const.tile([P, HPC, D], F32R, tag="m_all")
            for h in range(HPC):
                nc.tensor.matmul(pro_b[:, 1, h * P:(h + 1) * P],
                                 lhsT=ut_sb[:, h, :],
                                 rhs=w_sb["wvt"][:, h, :])
            nc.scalar.mul(m_all[:].rearrange("p h d -> p (h d)"),
                          pro_b[:, 1, :], SCALE)

            # ---- main loop over chunk PAIRS (one 2-bank PSUM tile each) ----
            m_flat = m_all[:].rearrange("p h d -> p (h d)")
            for j in range(NCH // 2):
                y2 = ps.tile([P, 2, NW], F32, tag="y2")
                for k in range(2):
                    i = 2 * j + k
                    nc.tensor.matmul(y2[:, k, :],
                                     lhsT=xt_sb[:, i * P:(i + 1) * P],
                                     rhs=m_flat, start=True, stop=False,
                                     skip_group_check=True)

                # one -max reduce for both chunks (all 8 (chunk,head) groups)
                negmax2 = work.tile([P, 2, HPC], BF16, tag="negmax2")
                nc.vector.reduce_max(
                    out=negmax2,
                    in_=y2[:].rearrange("p c (h d) -> p c h d", h=HPC),
                    axis=mybir.AxisListType.X, negate=True)

                # rank-1 subtract per chunk: y[s,(h,d)] += I * (-max bcast)
                for k in range(2):
                    nc.tensor.matmul(
                        y2[:, k, :].rearrange("p (h d) -> p h d", h=HPC),
                        lhsT=ident,
                        rhs=negmax2[:, k, :, None].to_broadcast((P, HPC, D)),
                        start=False, stop=True, skip_group_check=True)

                # one exp over both chunks (bias-free), bf16 out
                e2 = work.tile([P, 2, HPC, D], BF16, tag="e2")
                nc.scalar.activation(
                    e2, y2[:].rearrange("p c (h d) -> p c h d", h=HPC),
                    mybir.ActivationFunctionType.Exp)

                sums2 = work.tile([P, 2, HPC], F32, tag="sums2")
                nc.vector.reduce_sum(out=sums2, in_=e2,
                                     axis=mybir.AxisListType.X)
                rsum2 = work.tile([P, 2, HPC], F32, tag="rsum2")
                nc.vector.reciprocal(rsum2, sums2)

                o2 = work.tile([P, 2, HPC, D], BF16, tag="o2")
                # scalar engine takes two (chunk,head) slices of the multiply
                # (per-partition scale = rsum2[:,c,h]); Pool does the rest
                for c, h in ((0, 0), (1, 0)):
                    nc.scalar.mul(o2[:, c, h, :], e2[:, c, h, :],
                                  rsum2[:, c, h:h + 1])
                nc.gpsimd.tensor_tensor(
                    o2[:, :, 1:, :], e2[:, :, 1:, :],
                    rsum2[:, :, 1:, None].to_broadcast((P, 2, HPC - 1, D)),
                    mybir.AluOpType.mult)
                # one contiguous 256KB store per pair
                nc.sync.dma_start(
                    out_d.ap()[2 * j * P:(2 * j + 2) * P, :]
                        .rearrange("(c p) w -> p c w", p=P),
                    o2[:].rearrange("p c h d -> p c (h d)"))

    nc.compile()
    return nc


def _get_program():
    global _PROG
    if _PROG is None:
        _PROG = _build_program()
    return _PROG


def _make_in_maps(x, W_q, W_k, W_v):
    # pure host-side LAYOUT staging (transpose/slice only, no arithmetic)
    wvt_full = np.ascontiguousarray(
        W_v.reshape(H, D, D).transpose(0, 2, 1)).reshape(H * D, D)
    ident = np.eye(P, dtype=np.float32).astype(ml_dtypes.bfloat16)
    in_maps = []
    for core in range(N_CORES):
        b, hg = core // 2, core % 2
        sl = slice(hg * HPC * D, (hg + 1) * HPC * D)
        def stage(w):  # [(h p), c] -> [p, (h c)] to match the SBUF tile
            return np.ascontiguousarray(
                w.reshape(HPC, P, D).transpose(1, 0, 2).reshape(P, HPC * D))
        in_maps.append({
            "xb": np.ascontiguousarray(
                x[b].reshape(P, NCH * D).astype(ml_dtypes.bfloat16)),
            "xt": np.ascontiguousarray(x[b].T),
            "wq": stage(W_q[sl]),
            "wk": stage(W_k[sl]),
            "wvt": stage(wvt_full[sl]),
            "ident": ident,
        })
    return in_maps


def run(x, W_q, W_k, W_v, trace=False, **spmd_kwargs):
    """Run on 8 NeuronCores; returns (Z, BassKernelResults)."""
    nc = _get_program()
    in_maps = _make_in_maps(np.asarray(x, np.float32), np.asarray(W_q, np.float32),
                            np.asarray(W_k, np.float32), np.asarray(W_v, np.float32))
    res = run_bass_kernel_spmd(nc, in_maps, core_ids=list(range(N_CORES)),
                               trace=trace, **spmd_kwargs)
    Z = np.empty((B, H, S, D), np.float32)
    for core in range(N_CORES):
        b, hg = core // 2, core % 2
        o = np.asarray(res.results[core]["out"]).astype(np.float32)
        Z[b, hg * HPC:(hg + 1) * HPC] = o.reshape(S, HPC, D).transpose(1, 0, 2)
    return Z, res


def kernel(x, W_q, W_k, W_v):
    Z, _ = run(x, W_q, W_k, W_v, trace=False)
    return Z


# revision 25
# speedup vs baseline: 1.0702x; 1.0023x over previous
"""Trainium2 Bass kernel for nn_MHA_2688649527670.

Reference computes, per batch b and head h:
    Q = x Wq_h^T, K = x Wk_h^T, V = x Wv_h^T          ([S, D] each)
    Z = softmax_over_d( (Q K^T / sqrt(D)) V )

There is NO softmax between Q K^T and V, so the chain is associative:
    (Q K^T) V = x * (Wq_h^T Wk_h G Wv_h^T) / sqrt(D),   G = x^T x   ([D, D])

This collapses the O(S^2 D) attention into a [D,D] weight-chain plus one
[S,D]x[D,D] matmul per head, followed by softmax over the model dim.

Sharding: data parallel over batch (4) x tensor parallel over head-groups
(2 groups of 4 heads) = 8 cores, fully independent (no collectives).

Measured hardware facts driving the structure (this part is power-capped:
PE ~1.1GHz max; N=512 matmul ~460ns fp32r or bf16; LDWEIGHTS 395ns fp32r
vs 116ns bf16; DVE reduce ~680ns per op nearly independent of size up to
1024 elems/partition; Pool tensor_tensor ~1ns/elem; fp32r ~ 13-bit mantissa):
  * finals fp32r (bf16 finals measured 3.1e-2 error: too lossy).
  * G = x^T x fully bf16 (total error ~8e-3 vs 2e-2 budget).
  * per-(row,head) max subtract runs ON the PE as a rank-1 accumulating
    matmul: identity stationary (bf16, exact), -max broadcast over d moving.
    Softmax is shift-invariant so the bf16-rounded shift stays exact.
  * chunks processed in PAIRS sharing one 2-bank PSUM tile: one DVE max,
    one [128,1024] exp, one DVE sum, one reciprocal, one Pool multiply and
    one contiguous 256KB store per pair (fixed per-op costs dominate).
  * output stored as bf16 (0.2% quantization, halves store traffic);
    host upcasts to f32.
  * x^T / per-head Wv^T staged pre-transposed on the HOST (pure layout).
"""

import numpy as np
import ml_dtypes

import concourse.bass as bass
import concourse.bacc as bacc
import concourse.mybir as mybir
import concourse.tile as tile
from concourse.bass_utils import run_bass_kernel_spmd

B, S, D, H = 4, 2048, 128, 8
P = 128
HPC = H // 2          # heads per core (tensor parallel over 2 head groups)
NCH = S // P          # 16 s-chunks of 128 rows
NW = HPC * D          # 512 output columns per core
N_CORES = 8
SCALE = 1.0 / float(np.sqrt(D))
F32 = mybir.dt.float32
F32R = mybir.dt.float32r
BF16 = mybir.dt.bfloat16

_PROG = None  # cached compiled Bass program (same SPMD program for all cores)


def _build_program():
    nc = bacc.Bacc("TRN2", target_bir_lowering=False, debug=False,
                   num_devices=N_CORES)

    xb_d = nc.dram_tensor("xb", [P, NCH * D], BF16, kind="ExternalInput")
    xt_d = nc.dram_tensor("xt", [D, S], F32R, kind="ExternalInput")
    wq_d = nc.dram_tensor("wq", [P, HPC * D], F32R, kind="ExternalInput")
    wk_d = nc.dram_tensor("wk", [P, HPC * D], F32R, kind="ExternalInput")
    wvt_d = nc.dram_tensor("wvt", [P, HPC * D], F32R, kind="ExternalInput")
    id_d = nc.dram_tensor("ident", [P, P], BF16, kind="ExternalInput")
    out_d = nc.dram_tensor("out", [S, NW], BF16, kind="ExternalOutput")

    with tile.TileContext(nc) as tc:
        with (
            tc.tile_pool(name="ps", bufs=4, space="PSUM") as ps,
            tc.tile_pool(name="const", bufs=1) as const,
            tc.tile_pool(name="chain", bufs=2) as chain,
            tc.tile_pool(name="work", bufs=4) as work,
        ):
            # all PSUM flows through one ring of 4 two-bank pair tiles; the
            # prologue carves its small accumulators out of the first two
            # ring slots (dead before the pair loop wraps around to them)
            pro_a = ps.tile([P, 2, NW], F32, tag="y2")
            pro_b = ps.tile([P, 2, NW], F32, tag="y2")

            # PE p-state warm-up: the clock ramps only under continuous load,
            # and G lands on a cold PE otherwise. Values are never read.
            warm = const.tile([P, NW], BF16, tag="warm")
            nc.gpsimd.memset(warm, 1.0)
            for _ in range(12):
                nc.tensor.matmul(pro_a[:, 1, :], lhsT=warm[:, 0:P], rhs=warm)
            # weights are host-staged to the SBUF layout [p, h, c], so each
            # DMA is a plain [128, 512] contiguous 2D copy (2KB/partition)
            w_sb = {}
            for eng, nm, wd in ((nc.scalar, "wq", wq_d), (nc.scalar, "wk", wk_d),
                                (nc.scalar, "wvt", wvt_d)):
                t = const.tile([P, HPC, D], F32R, tag=f"{nm}_sb", name=f"{nm}_sb")
                eng.dma_start(t[:].rearrange("p h c -> p (h c)"), wd.ap())
                w_sb[nm] = t
            ident = const.tile([P, P], BF16, tag="ident")
            nc.scalar.dma_start(ident, id_d.ap())
            # x (bf16, for G) rides ALONE on the sync queue: DMAs sharing a
            # queue complete together, and G gates the whole prologue
            x_bf = const.tile([P, NCH, D], BF16, tag="x_bf")
            nc.sync.dma_start(x_bf[:].rearrange("p n c -> p (n c)"), xb_d.ap())
            # x^T for the finals, last on the scalar queue (needed latest)
            xt_sb = const.tile([P, S], F32R, tag="xt_sb")
            nc.scalar.dma_start(xt_sb, xt_d.ap())

            # preload the Exp activation table while DMAs stream
            pre_e = const.tile([P, 1], BF16, tag="pre_e")
            nc.scalar.activation(pre_e, ident[:, 0:1],
                                 mybir.ActivationFunctionType.Exp)

            # ---- chain part 1: P0T_h = (Wq_h^T Wk_h)^T (independent of G) ---
            p0t_all = const.tile([P, HPC, P], F32R, tag="p0t_all")
            for h in range(HPC):
                nc.tensor.matmul(pro_b[:, 0, h * P:(h + 1) * P],
                                 lhsT=w_sb["wk"][:, h, :],
                                 rhs=w_sb["wq"][:, h, :])
            nc.vector.tensor_copy(
                p0t_all[:].rearrange("p h c -> p (h c)"), pro_b[:, 0, :])

            # ---- G = x^T x in bf16 (error budget is ample; 3.4x faster) ----
            g_ps = pro_a[:, 0, 0:P]
            for i in range(NCH):
                nc.tensor.matmul(g_ps, lhsT=x_bf[:, i, :], rhs=x_bf[:, i, :],
                                 start=(i == 0), stop=(i == NCH - 1))
            g_sb = const.tile([P, P], F32R, tag="g_sb")
            nc.vector.tensor_copy(g_sb, g_ps)

            # ---- chain part 2: UT for all heads in ONE N=512 matmul,
            #      then M_h = UT_h^T WvT_h / sqrt(D) ----
            ut_ps = pro_a[:, 1, :]
            nc.tensor.matmul(ut_ps, lhsT=g_sb,
                             rhs=p0t_all[:].rearrange("p h c -> p (h c)"))
            ut_sb = chain.tile([P, HPC, P], F32R, tag="ut_sb")
            nc.vector.tensor_copy(ut_sb, ut_ps)

            m_all = const.tile([P, HPC, D], F32R, tag="m_all")
            for h in range(HPC):
                nc.tensor.matmul(pro_b[:, 1, h * P:(h + 1) * P],
                                 lhsT=ut_sb[:, h, :],
                                 rhs=w_sb["wvt"][:, h, :])
            nc.scalar.mul(m_all[:].rearrange("p h d -> p (h d)"),
                          pro_b[:, 1, :], SCALE)

            # ---- main loop over chunk PAIRS (one 2-bank PSUM tile each) ----
            m_flat = m_all[:].rearrange("p h d -> p (h d)")
            for j in range(NCH // 2):
                y2 = ps.tile([P, 2, NW], F32, tag="y2")
                for k in range(2):
                    i = 2 * j + k
                    nc.tensor.matmul(y2[:, k, :],
                                     lhsT=xt_sb[:, i * P:(i + 1) * P],
                                     rhs=m_flat, start=True, stop=False,
                                     skip_group_check=True)

                # one -max reduce for both chunks (all 8 (chunk,head) groups)
                negmax2 = work.tile([P, 2, HPC], BF16, tag="negmax2")
                nc.vector.reduce_max(
                    out=negmax2,
                    in_=y2[:].rearrange("p c (h d) -> p c h d", h=HPC),
                    axis=mybir.AxisListType.X, negate=True)

                # rank-1 subtract per chunk: y[s,(h,d)] += I * (-max bcast)
                for k in range(2):
                    nc.tensor.matmul(
                        y2[:, k, :].rearrange("p (h d) -> p h d", h=HPC),
                        lhsT=ident,
                        rhs=negmax2[:, k, :, None].to_broadcast((P, HPC, D)),
                        start=False, stop=True, skip_group_check=True)

                # one exp over both chunks (bias-free), bf16 out
                e2 = work.tile([P, 2, HPC, D], BF16, tag="e2")
                nc.scalar.activation(
                    e2, y2[:].rearrange("p c (h d) -> p c h d", h=HPC),
                    mybir.ActivationFunctionType.Exp)

                sums2 = work.tile([P, 2, HPC], F32, tag="sums2")
                nc.vector.reduce_sum(out=sums2, in_=e2,
                                     axis=mybir.AxisListType.X)
                rsum2 = work.tile([P, 2, HPC], F32, tag="rsum2")
                nc.vector.reciprocal(rsum2, sums2)

                o2 = work.tile([P, 2, HPC, D], BF16, tag="o2")
                # scalar engine takes two (chunk,head) slices of the multiply
                # (per-partition scale = rsum2[:,c,h]); Pool does the rest
                for c, h in ((0, 0), (1, 0)):
                    nc.scalar.mul(o2[:, c, h, :], e2[:, c, h, :],
                                  rsum2[:, c, h:h + 1])
                nc.gpsimd.tensor_tensor(
                    o2[:, :, 1:, :], e2[:, :, 1:, :],
                    rsum2[:, :, 1:, None].to_broadcast((P, 2, HPC - 1, D)),
                    mybir.AluOpType.mult)
                # one contiguous 256KB store per pair
                nc.sync.dma_start(
                    out_d.ap()[2 * j * P:(2 * j + 2) * P, :]
                        .rearrange("(c p) w -> p c w", p=P),
                    o2[:].rearrange("p c h d -> p c (h d)"))

    nc.compile()
    return nc


def _get_program():
    global _PROG
    if _PROG is None:
        _PROG = _build_program()
    return _PROG


def _make_in_maps(x, W_q, W_k, W_v):
    # pure host-side LAYOUT staging (transpose/slice only, no arithmetic)
    wvt_full = np.ascontiguousarray(
        W_v.reshape(H, D, D).transpose(0, 2, 1)).reshape(H * D, D)
    ident = np.eye(P, dtype=np.float32).astype(ml_dtypes.bfloat16)
    in_maps = []
    for core in range(N_CORES):
        b, hg = core // 2, core % 2
        sl = slice(hg * HPC * D, (hg + 1) * HPC * D)
        def stage(w):  # [(h p), c] -> [p, (h c)] to match the SBUF tile
            return np.ascontiguousarray(
                w.reshape(HPC, P, D).transpose(1, 0, 2).reshape(P, HPC * D))
        in_maps.append({
            "xb": np.ascontiguousarray(
                x[b].reshape(P, NCH * D).astype(ml_dtypes.bfloat16)),
            "xt": np.ascontiguousarray(x[b].T),
            "wq": stage(W_q[sl]),
            "wk": stage(W_k[sl]),
            "wvt": stage(wvt_full[sl]),
            "ident": ident,
        })
    return in_maps


def run(x, W_q, W_k, W_v, trace=False, **spmd_kwargs):
    """Run on 8 NeuronCores; returns (Z, BassKernelResults)."""
    nc = _get_program()
    in_maps = _make_in_maps(np.asarray(x, np.float32), np.asarray(W_q, np.float32),
                            np.asarray(W_k, np.float32), np.asarray(W_v, np.float32))
    res = run_bass_kernel_spmd(nc, in_maps, core_ids=list(range(N_CORES)),
                               trace=trace, **spmd_kwargs)
    Z = np.empty((B, H, S, D), np.float32)
    for core in range(N_CORES):
        b, hg = core // 2, core % 2
        o = np.asarray(res.results[core]["out"]).astype(np.float32)
        Z[b, hg * HPC:(hg + 1) * HPC] = o.reshape(S, HPC, D).transpose(1, 0, 2)
    return Z, res


def kernel(x, W_q, W_k, W_v):
    Z, _ = run(x, W_q, W_k, W_v, trace=False)
    return Z
